# revision 48
# baseline (speedup 1.0000x reference)
"""EpisodicMemory kernel for Trainium2, 8-core data-parallel, bf16 pipeline.

Reference computation (per batch b, d=32, m=64 memory slots, 2 hops):
    M = vs[b]
    for hop:
        Rh[m,:] = R[b,hop,m] @ h[b,hop,m]                  # batched matvec
        z = [Rh*v, Rh*M, |Rh-v|, |Rh-M|]                   # [m, 4d]
        Z = tanh(z @ W1.T + b1) @ W2.T (+ b2: dropped — softmax-invariant)
        g = softmax(Z over m); o = sum_m ts[b,hop,m] * g[m]
        M = GRUCell(o, M)
    out[b] = M

Sharding: pure data parallel over batch; 128 batches per core.

Strategy (vs f32 per-block baseline):
  - Rs/hs/ts host-cast to bf16: halves HBM traffic; DVE tensor_tensor runs
    in 2x_1p mode (2 elem/cycle) on 16-bit unit-stride operands.
  - einsum partitions = (bp, m); blocks processed in PAIRS (one 2 MiB
    R load as two 1 MiB DMAs, per-block muls, pair-fused add-tree and
    features) to halve instruction/semaphore count.
  - e-sum and the o m-sum as pairwise add trees on DVE (2x mode);
    TensorReduce has no fast DVE modes. GPSIMD measured ~10x slower than
    the cost model on bf16 tensor ops -> everything stays on DVE.
  - z computed TRANSPOSED on PE: mm2 becomes 4 tiny column matmuls
    (lhsT = a1 chunk, rhs = W2T) into a zcol PSUM tile; two 64x64 PE
    transposes per hop rebuild Z rows. No z DRAM bounce, no scatter DMAs;
    softmax runs in permuted batch order b' = (bp, blk, g), un-permuted
    on host.
  - softmax unnormalized (1/sum folded into o); GRU r,z gates PSUM-
    accumulate both matmuls then a single biased Sigmoid; n-gate uses a
    fused scalar_tensor_tensor.
  - MLP matmuls + feature transposes in bf16 (PE fp32 matmul = 4 cyc/row).
  - Host pre-computes permutes, W1T/W2T/gru_w/bias_pack/vb_pack packs,
    vs_rep, identity matrices (const loads: 8 DMAs on the ACT ring; the
    SP ring exclusively streams R).
  - M_rep rebuild between hops via PE broadcast matmul (bp_sel lhsT).
"""

import numpy as np
import ml_dtypes

import concourse.bacc as bacc
import concourse.bass as bass
import concourse.mybir as mybir
import concourse.tile as tile

F32 = mybir.dt.float32
BF16 = mybir.dt.bfloat16
AF = mybir.ActivationFunctionType
ALU = mybir.AluOpType
AX = mybir.AxisListType

B, N_HOP, N_MEM, DIM = 1024, 2, 64, 32
N_CORES = 8
BC = B // N_CORES            # 128 batches per core
BB = 8                       # batches per block
NBLK = BC // BB              # 16 blocks
NPAIR = NBLK // 2            # 8 block-pairs
NG = BB // 2                 # 4 b-pair groups per block
ROWS = BB * N_MEM            # 512 rows per block
D4 = 4 * DIM                 # 128 MLP input features
BLK_F = NG * DIM * DIM       # 4096 free elems per block in an R tile


def build_nc(n_iter: int = 1, feat_pool: bool = False, mul_pool: bool = False) -> bass.Bass:
    nc = bacc.Bacc("TRN2")

    # host-permuted: [hop, blk, bp, m, g, d, e] with b = blk*8+bp*4+g
    Rs_d = nc.dram_tensor(
        "Rs", [N_HOP, NBLK, 2, N_MEM, NG, DIM, DIM], BF16, kind="ExternalInput"
    )
    hs_d = nc.dram_tensor(
        "hs", [N_HOP, 2, N_MEM, NBLK, NG, DIM], BF16, kind="ExternalInput"
    )
    # ts host-permuted to [hop, b', d, m] with b' = bp*64 + blk*4 + g
    ts_d = nc.dram_tensor("ts", [N_HOP, BC, DIM, N_MEM], BF16, kind="ExternalInput")
    vs_rep_d = nc.dram_tensor(
        "vs_rep", [128, NBLK * NG * DIM], BF16, kind="ExternalInput"
    )
    identf_d = nc.dram_tensor("identf", [128, 128], F32, kind="ExternalInput")
    # cols 0-127: bf16 identity; cols 128-255 rows 0-1: bp_sel
    mpack_d = nc.dram_tensor("mpack", [128, 256], BF16, kind="ExternalInput")
    W1T_d = nc.dram_tensor("W1T", [D4, DIM], BF16, kind="ExternalInput")
    W2T_d = nc.dram_tensor("W2T", [DIM, 1], BF16, kind="ExternalInput")
    # gru_w[:, (hop, ih/hh, 3d)]: WihT/WhhT packed column-wise
    gruw_d = nc.dram_tensor(
        "gru_w", [DIM, N_HOP * 2 * 3 * DIM], F32, kind="ExternalInput"
    )
    # bias_pack[:, 3*hop+0]=(b_ih+b_hh), +1=b_ih, +2=b_hh  (96 partitions)
    biasp_d = nc.dram_tensor(
        "bias_pack", [DIM, 4 * N_HOP], F32, kind="ExternalInput"
    )
    # vb_pack: col 0 = b1, cols 1..129 = vsT (b'-ordered)
    vbp_d = nc.dram_tensor("vb_pack", [DIM, 1 + BC], F32, kind="ExternalInput")
    out_d = nc.dram_tensor("out", [BC, DIM], F32, kind="ExternalOutput")
    m_scr = nc.dram_tensor("m_scratch", [BC, DIM], BF16)  # internal DRAM bounce

    import contextlib

    with tile.TileContext(nc) as tc:
        with contextlib.ExitStack() as stk:
            # consts pool + loads sit OUTSIDE the For_i: loop-invariant
            # weights/identities load once, not per timed iteration
            consts = stk.enter_context(tc.tile_pool(name="consts", bufs=1))
            ident = consts.tile([128, 128], F32)
            nc.scalar.dma_start(out=ident, in_=identf_d[:, :])
            mpack = consts.tile([128, 256], BF16)
            nc.scalar.dma_start(out=mpack, in_=mpack_d[:, :])
            ident_bf = mpack[:, 0:128]

            # ---- weights (packed, on the ACT ring; SP ring streams R) ----
            W1T = consts.tile([D4, DIM], BF16)
            nc.scalar.dma_start(out=W1T, in_=W1T_d[:, :])
            W2T = consts.tile([DIM, 1], BF16)
            nc.scalar.dma_start(out=W2T, in_=W2T_d[:, :])
            gruw = consts.tile([DIM, N_HOP * 2 * 3 * DIM], F32)
            nc.scalar.dma_start(out=gruw, in_=gruw_d[:, :])
            biasp = consts.tile([DIM, 4 * N_HOP], F32)
            nc.scalar.dma_start(out=biasp, in_=biasp_d[:, :])
            vbp = consts.tile([DIM, 1 + BC], F32)
            nc.scalar.dma_start(out=vbp, in_=vbp_d[:, :])
            b1T = vbp[:, 0:1]
            WihT = [
                gruw[:, (2 * hop) * 3 * DIM : (2 * hop + 1) * 3 * DIM]
                for hop in range(N_HOP)
            ]
            WhhT = [
                gruw[:, (2 * hop + 1) * 3 * DIM : (2 * hop + 2) * 3 * DIM]
                for hop in range(N_HOP)
            ]
            # bias_pack cols per hop: 0=r_sum, 1=z_sum, 2=b_ih_n, 3=b_hh_n
            # (all at partition base 0; only rows 0:32 used)
            bsum_rz = [
                (
                    biasp[0:DIM, 4 * hop : 4 * hop + 1],
                    biasp[0:DIM, 4 * hop + 1 : 4 * hop + 2],
                )
                for hop in range(N_HOP)
            ]
            bihn_t = [
                biasp[0:DIM, 4 * hop + 2 : 4 * hop + 3] for hop in range(N_HOP)
            ]
            bhhn_t = [
                biasp[0:DIM, 4 * hop + 3 : 4 * hop + 4] for hop in range(N_HOP)
            ]

            # ---- initial M state ----
            MT = vbp[:, 1:]  # current M^T [d, b']

            v_rep = consts.tile([128, NBLK * NG * DIM], BF16)
            nc.scalar.dma_start(out=v_rep, in_=vs_rep_d[:, :])
            M_rep = v_rep  # hop 0: M == vs

            bp_sel = mpack[0:2, 128:256]

            # ---- timed region: For_i + per-iteration pools ----
            if n_iter > 1:
                stk.enter_context(tc.For_i(0, n_iter, 1))
            hop_io = stk.enter_context(tc.tile_pool(name="hop_io", bufs=4))
            rpool = stk.enter_context(tc.tile_pool(name="rpool", bufs=5))
            tpool = stk.enter_context(tc.tile_pool(name="tpool", bufs=2))
            fpool = stk.enter_context(tc.tile_pool(name="fpool", bufs=3))
            zpool = stk.enter_context(tc.tile_pool(name="zpool", bufs=3))
            apool = stk.enter_context(tc.tile_pool(name="apool", bufs=3))
            small = stk.enter_context(tc.tile_pool(name="small", bufs=2))
            mstate = stk.enter_context(tc.tile_pool(name="mstate", bufs=2))
            pp_z = stk.enter_context(
                tc.tile_pool(name="pp_z", bufs=1, space="PSUM"))
            pp_1 = stk.enter_context(
                tc.tile_pool(name="pp_1", bufs=2, space="PSUM"))
            pp_2 = stk.enter_context(
                tc.tile_pool(name="pp_2", bufs=1, space="PSUM"))
            pp_g = stk.enter_context(
                tc.tile_pool(name="pp_g", bufs=2, space="PSUM"))

            # per-hop h/t loads: first muls need h_hop
            h_hops, t_hops = [], []
            PSLC = 2 * NG * DIM  # one pair's h columns
            for hop in range(N_HOP):
                h_hop = hop_io.tile([128, NBLK * NG * DIM], BF16, tag="h_hop")
                h_src = hs_d[hop].rearrange("bp m blk g e -> (bp m) (blk g e)")
                nc.scalar.dma_start(
                    out=h_hop[:, :PSLC], in_=h_src[:, :PSLC]
                )
                nc.scalar.dma_start(
                    out=h_hop[:, PSLC:], in_=h_src[:, PSLC:]
                )
                t_hop = hop_io.tile([BC, DIM * N_MEM], BF16, tag="t_hop")
                nc.scalar.dma_start(
                    out=t_hop, in_=ts_d[hop].rearrange("b d m -> b (d m)")
                )
                h_hops.append(h_hop)
                t_hops.append(t_hop)

            for hop in range(N_HOP):
                h_hop = h_hops[hop]
                t_hop = t_hops[hop]

                # z columns accumulate here: [part=(bp,m), col=(blk,g)]
                Z2 = small.tile([128, NBLK * NG], F32, tag="Z2")

                for pr in range(NPAIR):
                    blk0 = 2 * pr
                    r2 = rpool.tile([128, 2 * BLK_F], BF16, tag="R")
                    for bk in range(2):
                        nc.sync.dma_start(
                            out=r2[:, bk * BLK_F : (bk + 1) * BLK_F],
                            in_=Rs_d[hop, blk0 + bk].rearrange(
                                "bp m g d e -> (bp m) (g d e)"
                            ),
                        )
                    # P = R * h (in-place), h broadcast over d (middle dim).
                    # One block's mul on DVE, the other on GPSIMD.
                    h2 = (
                        h_hop[:, blk0 * NG * DIM : (blk0 + 2) * NG * DIM]
                        .rearrange("p (bk g e) -> p bk g e", bk=2, g=NG)
                        .unsqueeze(3)
                        .broadcast_to((128, 2, NG, DIM, DIM))
                    )
                    r4 = r2.rearrange(
                        "p (bk g d e) -> p bk g d e", bk=2, g=NG, d=DIM
                    )
                    if mul_pool and pr < NPAIR - 1:
                        nc.gpsimd.tensor_tensor(
                            r4[:, 0], r4[:, 0], h2[:, 0], op=ALU.mult
                        )
                        nc.vector.tensor_mul(r4[:, 1], r4[:, 1], h2[:, 1])
                    else:
                        # per-block muls: each starts as soon as its half of
                        # the R pair-load lands
                        nc.vector.tensor_mul(r4[:, 0], r4[:, 0], h2[:, 0])
                        nc.vector.tensor_mul(r4[:, 1], r4[:, 1], h2[:, 1])

                    # Rh[(bp,m), (bk,g,d)] = sum_e P:
                    # 5-level pairwise add tree over e (bf16 2x mode on DVE)
                    rh = fpool.tile([128, 2 * NG * DIM], BF16, tag="rh")
                    tscr = tpool.tile([128, 7680], BF16, tag="tree")
                    lv_in = r2.rearrange("p (gd e) -> p gd e", e=DIM)
                    off = 0
                    w = DIM // 2
                    for lv in range(5):
                        if lv == 4:
                            out_ap = rh.rearrange("p (gd e) -> p gd e", e=1)
                        else:
                            out_ap = tscr[:, off : off + 256 * w].rearrange(
                                "p (gd e) -> p gd e", e=w
                            )
                        nc.vector.tensor_add(
                            out_ap, lv_in[:, :, :w], lv_in[:, :, w : 2 * w]
                        )
                        lv_in = out_ap
                        off += 256 * w
                        w //= 2

                    # features F [(bp,m), (bk, g, f, d)]
                    f_blk = fpool.tile([128, 2 * NG * 4 * DIM], BF16, tag="F")
                    f4 = f_blk.rearrange(
                        "p (bk g f d) -> p bk g f d", bk=2, g=NG, f=4
                    )
                    rh3 = rh.rearrange("p (bk g d) -> p bk g d", bk=2, g=NG)
                    vr3 = v_rep[
                        :, blk0 * NG * DIM : (blk0 + 2) * NG * DIM
                    ].rearrange("p (bk g d) -> p bk g d", bk=2, g=NG)
                    mr3 = M_rep[
                        :, blk0 * NG * DIM : (blk0 + 2) * NG * DIM
                    ].rearrange("p (bk g d) -> p bk g d", bk=2, g=NG)
                    feng = nc.gpsimd if feat_pool else nc.vector
                    feng.tensor_tensor(
                        f4[:, :, :, 0, :], rh3, vr3, op=ALU.mult
                    )
                    feng.tensor_tensor(
                        f4[:, :, :, 1, :], rh3, mr3, op=ALU.mult
                    )
                    feng.tensor_tensor(
                        f4[:, :, :, 2, :], rh3, vr3, op=ALU.subtract
                    )
                    feng.tensor_tensor(
                        f4[:, :, :, 3, :], rh3, mr3, op=ALU.subtract
                    )
                    nc.scalar.activation(f4[:, :, :, 2, :], f4[:, :, :, 2, :], AF.Abs)
                    nc.scalar.activation(f4[:, :, :, 3, :], f4[:, :, :, 3, :], AF.Abs)

                    # transpose to z^T [(f,d), (bk,g,bp,m)]
                    zt_ps = pp_z.tile([D4, 2 * ROWS], BF16, tag="zt")
                    for c in range(2 * NG):
                        nc.tensor.transpose(
                            zt_ps[:, c * 128 : (c + 1) * 128],
                            f_blk[:, c * 128 : (c + 1) * 128],
                            ident_bf,
                        )
                    zt_sb = zpool.tile([D4, 2 * ROWS], BF16, tag="zt_sb")
                    nc.scalar.copy(out=zt_sb, in_=zt_ps)

                    zcol = pp_2.tile([128, 2 * NG], F32, tag="zcol")
                    for bk in range(2):
                        ps1 = pp_1.tile([DIM, ROWS], F32, tag="ps1")
                        nc.tensor.matmul(
                            ps1,
                            lhsT=W1T,
                            rhs=zt_sb[:, bk * ROWS : (bk + 1) * ROWS],
                            start=True,
                            stop=True,
                        )
                        a1 = apool.tile([DIM, ROWS], BF16, tag="a1")
                        nc.scalar.activation(a1, ps1, AF.Tanh, bias=b1T)
                        # z^T columns directly: out[(bp,m), 1] = a1_chunk^T @ W2T
                        for g in range(NG):
                            nc.tensor.matmul(
                                zcol[:, bk * NG + g : bk * NG + g + 1],
                                lhsT=a1[:, g * 128 : (g + 1) * 128],
                                rhs=W2T,
                                start=True,
                                stop=True,
                            )
                    nc.scalar.copy(
                        out=Z2[:, pr * 2 * NG : (pr + 1) * 2 * NG], in_=zcol
                    )

                # Z2 [(bp,m), (blk,g)] -> Z_row [b'=(bp,blk,g), m] via two
                # 64x64 PE transposes (per bp half)
                Z_row = small.tile([BC, N_MEM], F32, tag="Z_row")
                for bp in range(2):
                    ztr = pp_2.tile([N_MEM, N_MEM], F32, tag="ztr")
                    nc.tensor.transpose(
                        ztr,
                        Z2[bp * N_MEM : (bp + 1) * N_MEM],
                        ident[
                            bp * N_MEM : (bp + 1) * N_MEM,
                            bp * N_MEM : (bp + 1) * N_MEM,
                        ],
                    )
                    nc.scalar.copy(
                        out=Z_row[bp * N_MEM : (bp + 1) * N_MEM], in_=ztr
                    )

                # softmax over m (unnormalized: the 1/sum folds into o)
                nmx = small.tile([BC, 1], F32, tag="nmx")
                nc.vector.tensor_reduce(
                    out=nmx, in_=Z_row, axis=AX.X, op=ALU.max, negate=True
                )
                e_bf = small.tile([BC, N_MEM], BF16, tag="e_bf")
                nc.scalar.activation(e_bf, Z_row, AF.Exp, bias=nmx)
                ssum = small.tile([BC, 1], F32, tag="ssum")
                nc.vector.tensor_reduce(out=ssum, in_=e_bf, axis=AX.X, op=ALU.add)
                rsum = small.tile([BC, 1], F32, tag="rsum")
                nc.vector.reciprocal(rsum, ssum)

                # o[b,d] = (sum_m t[b,d,m] * e[b,m]) / sum_m e[b,m]
                t3 = t_hop.rearrange("b (d m) -> b d m", d=DIM)
                g3 = e_bf.unsqueeze(1).broadcast_to((BC, DIM, N_MEM))
                nc.vector.tensor_mul(t3, t3, g3)
                # m-sum as a pairwise add tree (2x mode) instead of a 1x
                # TensorReduce
                oscr = tpool.tile([128, 7680], BF16, tag="tree")
                o_row = small.tile([BC, DIM], F32, tag="o_row")
                lv_in = t3
                off = 0
                w = N_MEM // 2
                while w >= 1:
                    if w == 1:
                        out_ap = o_row.unsqueeze(2)
                    else:
                        out_ap = oscr[:, off : off + DIM * w].rearrange(
                            "b (d m) -> b d m", d=DIM
                        )
                    nc.vector.tensor_add(
                        out_ap, lv_in[:, :, :w], lv_in[:, :, w : 2 * w]
                    )
                    lv_in = out_ap
                    off += DIM * w
                    w //= 2
                nc.vector.tensor_scalar_mul(o_row, o_row, rsum)

                # GRU (transposed layout [*, b], f32)
                ot_ps = pp_g.tile([DIM, BC], F32, tag="gpsum")
                nc.tensor.transpose(ot_ps, o_row, ident)
                oT = small.tile([DIM, BC], F32, tag="oT")
                nc.scalar.copy(out=oT, in_=ot_ps)

                # r,z gates: both matmuls PSUM-accumulate, then one Sigmoid
                rz_t = []
                for g in range(2):
                    gacc = pp_g.tile([DIM, BC], F32, tag="gpsum")
                    nc.tensor.matmul(
                        gacc,
                        lhsT=WihT[hop][:, g * DIM : (g + 1) * DIM],
                        rhs=oT,
                        start=True,
                        stop=False,
                    )
                    nc.tensor.matmul(
                        gacc,
                        lhsT=WhhT[hop][:, g * DIM : (g + 1) * DIM],
                        rhs=MT,
                        start=False,
                        stop=True,
                    )
                    gt = small.tile([DIM, BC], F32, tag=f"gate{g}")
                    nc.scalar.activation(
                        gt, gacc, AF.Sigmoid, bias=bsum_rz[hop][g]
                    )
                    rz_t.append(gt)
                r_t, z_t = rz_t

                # n = tanh(gi_n + b_ih_n + r * (gh_n + b_hh_n))
                gi_n = pp_g.tile([DIM, BC], F32, tag="gpsum")
                nc.tensor.matmul(
                    gi_n, lhsT=WihT[hop][:, 2 * DIM :], rhs=oT,
                    start=True, stop=True,
                )
                gh_n = pp_g.tile([DIM, BC], F32, tag="gpsum")
                nc.tensor.matmul(
                    gh_n, lhsT=WhhT[hop][:, 2 * DIM :], rhs=MT,
                    start=True, stop=True,
                )
                n1 = small.tile([DIM, BC], F32, tag="n1")
                nc.vector.scalar_tensor_tensor(
                    out=n1, in0=gh_n, scalar=bhhn_t[hop], in1=r_t,
                    op0=ALU.add, op1=ALU.mult,
                )
                gin = small.tile([DIM, BC], F32, tag="gin")
                nc.scalar.activation(gin, gi_n, AF.Identity, bias=bihn_t[hop])
                nc.vector.tensor_add(n1, n1, gin)
                n_t = small.tile([DIM, BC], F32, tag="n_t")
                nc.scalar.activation(n_t, n1, AF.Tanh)

                # M' = n + z * (M - n)
                MT_new = mstate.tile([DIM, BC], F32, tag="MT")
                nc.vector.tensor_sub(MT_new, MT, n_t)
                nc.vector.tensor_mul(MT_new, MT_new, z_t)
                nc.vector.tensor_add(MT_new, MT_new, n_t)
                MT = MT_new

                # M_row for output / M_rep rebuild
                mrow_ps = pp_g.tile([BC, DIM], F32, tag="gpsum")
                nc.tensor.transpose(mrow_ps, MT, ident[:DIM, :DIM])
                M_row = mstate.tile([BC, DIM], F32, tag="M_row")
                nc.scalar.copy(out=M_row, in_=mrow_ps)

                if hop < N_HOP - 1:
                    # rebuild M_rep: SBUF->SBUF regroup + PE broadcast matmul
                    M_row_bf = mstate.tile([BC, DIM], BF16, tag="M_row_bf")
                    nc.scalar.copy(out=M_row_bf, in_=mrow_ps)
                    m_flat = mstate.tile([2, NBLK * NG * DIM], BF16, tag="m_flat")
                    nc.scalar.dma_start(out=m_scr[:, :], in_=M_row_bf)
                    nc.scalar.dma_start(
                        out=m_flat,
                        in_=m_scr.rearrange("(bp bg) d -> bp (bg d)", bp=2),
                    )
                    M_rep_new = mstate.tile(
                        [128, NBLK * NG * DIM], BF16, tag="M_rep", bufs=1
                    )
                    half = NBLK * NG * DIM // 4
                    for hf in range(4):
                        mrep_ps = pp_2.tile([128, half], F32, tag="mrep_ps")
                        nc.tensor.matmul(
                            mrep_ps,
                            lhsT=bp_sel,
                            rhs=m_flat[:, hf * half : (hf + 1) * half],
                            start=True,
                            stop=True,
                        )
                        nc.scalar.copy(
                            out=M_rep_new[:, hf * half : (hf + 1) * half],
                            in_=mrep_ps,
                        )
                    M_rep = M_rep_new
                else:
                    nc.scalar.dma_start(out=out_d[:, :], in_=M_row)

    nc.compile()
    return nc


_NC_CACHE = None


def _get_nc():
    global _NC_CACHE
    if _NC_CACHE is None:
        _NC_CACHE = build_nc()
    return _NC_CACHE


BF = ml_dtypes.bfloat16

# b = blk*8 + bp*4 + g  <->  b' = bp*64 + blk*4 + g  (softmax/GRU row order)
_b = np.arange(BC)
_blk, _bp, _g = _b // BB, (_b % BB) // NG, _b % NG
B_TO_BPRIME = _bp * 64 + _blk * NG + _g          # PERM[b] = b'
BPRIME_TO_B = np.argsort(B_TO_BPRIME)            # rows: b' -> b


def permute_local(x):
    """[BC, N_HOP, m, ...] -> [N_HOP, NBLK, 2, m, NG, ...], b = blk*8+bp*4+g."""
    tail = x.shape[2:]
    y = x.reshape(NBLK, 2, NG, N_HOP, *tail)  # [blk, bp, g, hop, m, ...]
    order = (3, 0, 1, 4, 2) + tuple(range(5, y.ndim))
    return np.ascontiguousarray(y.transpose(order))


def permute_h(x):
    """hs [BC, N_HOP, m, e] -> [N_HOP, 2, m, NBLK, NG, e]."""
    y = x.reshape(NBLK, 2, NG, N_HOP, N_MEM, DIM)
    return np.ascontiguousarray(y.transpose(3, 1, 4, 0, 2, 5))


def make_in_maps(hs, Rs, ts, vs, W1, b1, W2, W_ih, W_hh, b_ih, b_hh):
    W1T = np.ascontiguousarray(W1.T.astype(BF))
    W2T = np.ascontiguousarray(W2.T.astype(BF))
    WihT = W_ih.transpose(0, 2, 1)  # [hop, 32, 96]
    WhhT = W_hh.transpose(0, 2, 1)
    gru_w = np.ascontiguousarray(
        np.concatenate(
            [x for hop in range(N_HOP) for x in (WihT[hop], WhhT[hop])], axis=1
        )
    )
    bsum = b_ih + b_hh  # [hop, 96]
    bias_pack = np.stack(
        [
            col
            for hop in range(N_HOP)
            for col in (
                bsum[hop, 0:DIM],
                bsum[hop, DIM : 2 * DIM],
                b_ih[hop, 2 * DIM :],
                b_hh[hop, 2 * DIM :],
            )
        ],
        axis=1,
    )  # [32, 4*hop]
    in_maps = []
    for c in range(N_CORES):
        sl = slice(c * BC, (c + 1) * BC)
        vsc = vs[sl]
        # vs_rep[(bp,m), (blk,g,d)] = vs[blk*8+bp*4+g, d]
        v4 = vsc.reshape(NBLK, 2, NG, DIM).transpose(1, 0, 2, 3)  # [bp,blk,g,d]
        vs_rep = np.broadcast_to(
            v4.reshape(2, 1, NBLK * NG * DIM), (2, N_MEM, NBLK * NG * DIM)
        ).reshape(128, NBLK * NG * DIM)
        identf = np.eye(128, dtype=np.float32)
        mpack = np.zeros((128, 256), dtype=BF)
        mpack[:, 0:128] = np.eye(128, dtype=np.float32).astype(BF)
        sel = np.zeros((2, 128), dtype=np.float32)
        sel[0, 0:N_MEM] = 1.0
        sel[1, N_MEM:128] = 1.0
        mpack[0:2, 128:256] = sel.astype(BF)
        in_maps.append(
            {
                "identf": identf,
                "mpack": mpack,
                "Rs": permute_local(Rs[sl]).astype(BF),
                "hs": permute_h(hs[sl]).astype(BF),
                "ts": np.ascontiguousarray(
                    ts[sl].transpose(1, 0, 3, 2)[:, BPRIME_TO_B]
                ).astype(BF),
                "vb_pack": np.ascontiguousarray(
                    np.concatenate(
                        [b1[:, None], vsc.T[:, BPRIME_TO_B]], axis=1
                    )
                ),
                "vs_rep": np.ascontiguousarray(vs_rep).astype(BF),
                "W1T": W1T,
                "W2T": W2T,
                "gru_w": gru_w,
                "bias_pack": np.ascontiguousarray(bias_pack),
            }
        )
    return in_maps


def kernel(hs, Rs, ts, vs, W1, b1, W2, b2, W_ih, W_hh, b_ih, b_hh):
    from concourse.bass_utils import run_bass_kernel_spmd

    nc = _get_nc()
    in_maps = make_in_maps(hs, Rs, ts, vs, W1, b1, W2, W_ih, W_hh, b_ih, b_hh)
    res = run_bass_kernel_spmd(nc, in_maps, list(range(N_CORES)))
    # kernel emits rows in b' order; restore natural batch order
    return np.concatenate(
        [r["out"][B_TO_BPRIME] for r in res.results], axis=0
    )
